# revision 1
# baseline (speedup 1.0000x reference)
"""Trainium2 Bass kernel for nn_CombineUV (shortlist-scored retrieval).

Math: out[b,s] = dot(input[b], sig(alpha)*weight[i] + sig(beta)*labels[i]) + bias[i]
with i = shortlist[b,s].  Folding the sigmoid gates into the input side:
out[b,s] = dot(xa[b], weight[i]) + dot(xb[b], labels[i]) + bias[i]
where xa = input*sig(alpha), xb = input*sig(beta) -- so the [L,D] combined
table is never materialized.

Device strategy (8 cores, L-sharded, stream+gather hybrid):
 - Combined table TC = [weight || labels] as [L, 1024] bf16; core c owns rows
   [c*16384, (c+1)*16384) so local indices fit dma_gather's int16 limit.
 - Each (b,s) pair is routed to the core owning its row. Per core, one pair
   per distinct row is served by a STREAM: the host pre-transposes those rows
   (sorted by the pair's batch) into PE-ready [128, 8*512] tiles that load
   with a plain full-rate dma_start (no SWDGE descriptor-gen cost). The
   remaining pairs (duplicate hits of a row) are served by
   dma_gather(transpose=True), which delivers the same tile layout:
   g[p, c*512+j] = TC[row_j, c*128+p].
 - Per 512-pair tile: 8 accumulating matmuls with lhsT = XC[:, c, b_lo:b_lo+64]
   (xa/xb chunks for a 64-wide batch window covering the tile) give
   PSUM[m, j] = xa[b_lo+m].W[i_j] + xb[b_lo+m].V[i_j]; a host-built one-hot
   mask (selects m_j = b_j - b_lo per column) is multiplied in on the vector
   engine, then a ones-vector matmul reduces partitions to the final score.
 - Host adds bias[shortlist] (O(B*S) elementwise) and inverse-permutes.
"""

import sys

sys.path.insert(0, "/opt/trn_rl_repo")

import numpy as np
import ml_dtypes

BF16 = ml_dtypes.bfloat16

L, D, B, S = 131072, 512, 512, 512
NCORES = 8
LSH = L // NCORES          # table rows per core (16384 -> int16-safe indices)
TILE = 512                 # pairs per tile
MWIN = 128                 # batch-window width for the lhsT slice
NCHUNK = (2 * D) // 128    # 8 chunks of 128 along the combined-row axis
ROW_ELEMS = 2 * D          # combined row length (bf16 elements)

_PROG_CACHE = {}


def _window_schedule(bvals_per_core, ntiles):
    """Joint (all-core) per-tile batch-window base. bvals_per_core[c] is the
    per-core padded [ntiles*TILE] batch array with -1 on padding slots.
    Returns blo [ntiles] or None if some tile cannot fit a MWIN-wide window."""
    blo = np.zeros(ntiles, np.int64)
    for t in range(ntiles):
        lo, hi = B, -1
        for bv in bvals_per_core:
            seg = bv[t * TILE : (t + 1) * TILE]
            seg = seg[seg >= 0]
            if len(seg):
                lo = min(lo, int(seg.min()))
                hi = max(hi, int(seg.max()))
        if hi < 0:
            lo, hi = 0, 0
        if hi - lo >= MWIN:
            return None
        blo[t] = min(lo, B - MWIN)
    return blo


def _tile_order(nstream, ngather):
    """Interleave: uniform mix of stream/gather tiles, but hold back the last
    few stream tiles for the end of the schedule — a stream tail paces at
    ~2.4-2.6us/tile (DMA/PE) while a gather tail is throttled by the serial
    Q7 descriptor-gen at ~4.7us/tile."""
    # Empirically (3 experiments) the uniform interleave beats any front-load
    # or reserved-tail variant: the Tile scheduler's dynamic slot recycling
    # paces the tail at compute speed regardless of DMA kind, and a uniform
    # mix keeps every engine fed throughout. tail_s=0 == uniform.
    tail_s = 0
    body_s = nstream - tail_s
    order = []
    si = gi = 0
    for t in range(body_s + ngather):
        take_stream = si < body_s and (gi >= ngather or si * ngather <= gi * body_s)
        if take_stream:
            order.append(("s", si))
            si += 1
        else:
            order.append(("g", gi))
            gi += 1
    for k in range(body_s, nstream):
        order.append(("s", k))
    return order


def _build_program(nstream, ngather, blo, cap_g):
    import concourse.bacc as bacc
    import concourse.mybir as mybir
    from concourse.tile import TileContext

    f32, bf, i16 = mybir.dt.float32, mybir.dt.bfloat16, mybir.dt.int16
    ntiles = nstream + ngather

    nc = bacc.Bacc(None, target_bir_lowering=False)
    tc_d = nc.dram_tensor("tc", [LSH, ROW_ELEMS], bf, kind="ExternalInput")
    st_d = nc.dram_tensor(
        "stream", [max(nstream, 1), 128, NCHUNK * TILE], bf, kind="ExternalInput"
    )
    xc_d = nc.dram_tensor("xc", [128, NCHUNK * B], bf, kind="ExternalInput")
    idx_d = nc.dram_tensor("idx", [128, max(cap_g, 16) // 16], i16, kind="ExternalInput")
    u8 = mybir.dt.uint8
    mask_d = nc.dram_tensor("mask", [MWIN, ntiles * TILE], u8, kind="ExternalInput")
    mask2_d = nc.dram_tensor(
        "mask2", [MWIN, max(nstream, 1) * TILE], u8, kind="ExternalInput"
    )
    ones_d = nc.dram_tensor("ones", [MWIN, 1], bf, kind="ExternalInput")
    out_d = nc.dram_tensor("out", [ntiles, TILE], f32, kind="ExternalOutput")
    out2_d = nc.dram_tensor(
        "out2", [max(nstream, 1), TILE], f32, kind="ExternalOutput"
    )

    order = _tile_order(nstream, ngather)

    with TileContext(nc) as tc:
        with (
            tc.tile_pool(name="res", bufs=1) as res_pool,
            tc.tile_pool(name="g", bufs=6) as gpool,
            tc.tile_pool(name="m", bufs=4) as mpool,
            tc.tile_pool(name="o", bufs=4) as opool,
            tc.tile_pool(name="ps", bufs=4, space="PSUM") as pspool,
            tc.tile_pool(name="ps2", bufs=2, space="PSUM") as ps2pool,
        ):
            xc_sb = res_pool.tile([128, NCHUNK * B], bf, tag="xc")
            nc.sync.dma_start(out=xc_sb[:], in_=xc_d[:])
            idx_sb = res_pool.tile([128, max(cap_g, 16) // 16], i16, tag="idx")
            nc.sync.dma_start(out=idx_sb[:], in_=idx_d[:])
            mask_sb = res_pool.tile([MWIN, ntiles * TILE], u8, tag="mask")
            nc.sync.dma_start(out=mask_sb[:], in_=mask_d[:])
            mask2_sb = res_pool.tile([MWIN, max(nstream, 1) * TILE], u8, tag="mask2")
            nc.sync.dma_start(out=mask2_sb[:], in_=mask2_d[:])
            ones_sb = res_pool.tile([MWIN, 1], bf, tag="ones")
            nc.sync.dma_start(out=ones_sb[:], in_=ones_d[:])

            for t, (kind, k) in enumerate(order):
                bl = int(blo[t])
                g = gpool.tile([128, NCHUNK * TILE], bf, tag="g")
                if kind == "s":
                    nc.sync.dma_start(out=g[:], in_=st_d[k])
                else:
                    g3 = g[:].rearrange("p (c j) -> p c j", j=TILE)
                    nc.gpsimd.dma_gather(
                        g3,
                        tc_d[:],
                        idx_sb[:, k * (TILE // 16) : (k + 1) * (TILE // 16)],
                        TILE,
                        TILE,
                        ROW_ELEMS,
                        transpose=True,
                    )
                ps = pspool.tile([MWIN, TILE], f32, tag="ps")
                for c in range(NCHUNK):
                    nc.tensor.matmul(
                        out=ps[:],
                        lhsT=xc_sb[:, c * B + bl : c * B + bl + MWIN],
                        rhs=g[:, c * TILE : (c + 1) * TILE],
                        start=(c == 0),
                        stop=(c == NCHUNK - 1),
                    )
                msk = mpool.tile([MWIN, TILE], bf, tag="msk")
                nc.vector.tensor_tensor(
                    out=msk[:],
                    in0=ps[:],
                    in1=mask_sb[:, t * TILE : (t + 1) * TILE],
                    op=mybir.AluOpType.mult,
                )
                ps2 = ps2pool.tile([1, TILE], f32, tag="ps2")
                nc.tensor.matmul(
                    out=ps2[:], lhsT=ones_sb[:], rhs=msk[:], start=True, stop=True
                )
                ot = opool.tile([1, TILE], f32, tag="ot")
                nc.scalar.copy(ot[:], ps2[:])
                nc.sync.dma_start(out=out_d[t : t + 1, :], in_=ot[:])
                if kind == "s":
                    # Second select pass: serves one extra pair per column
                    # whose batch also falls in this tile's window — these
                    # pairs cost no additional DMA at all.
                    msk2 = mpool.tile([MWIN, TILE], bf, tag="msk2")
                    nc.vector.tensor_tensor(
                        out=msk2[:],
                        in0=ps[:],
                        in1=mask2_sb[:, k * TILE : (k + 1) * TILE],
                        op=mybir.AluOpType.mult,
                    )
                    ps2b = ps2pool.tile([1, TILE], f32, tag="ps2b")
                    nc.tensor.matmul(
                        out=ps2b[:], lhsT=ones_sb[:], rhs=msk2[:], start=True, stop=True
                    )
                    ot2 = opool.tile([1, TILE], f32, tag="ot2")
                    nc.scalar.copy(ot2[:], ps2b[:])
                    nc.sync.dma_start(out=out2_d[k : k + 1, :], in_=ot2[:])

    # Bacc.compile splits multi-sem waits (HW allows 1/inst), auto-inserts
    # gpsimd library loads for dma_gather, and codegens ISA-subclass insts.
    nc.compile()
    return nc, order


def _prep_inputs(input, labels, weight, alpha, beta, shortlist, force_gather=False):
    """Host-side staging: sigmoid fold, bf16 casts, pair routing (stream vs
    gather), stream-table pre-transpose, mask build. With force_gather, every
    pair goes through the dma_gather path (fallback when the stream batch
    windows don't fit)."""
    input = np.asarray(input, dtype=np.float32)
    alpha = np.asarray(alpha, dtype=np.float32).reshape(1, D)
    beta = np.asarray(beta, dtype=np.float32).reshape(1, D)
    xa = input * (1.0 / (1.0 + np.exp(-alpha)))
    xb = input * (1.0 / (1.0 + np.exp(-beta)))

    # XC[p, c, b]: chunk c of xa (c<4) / xb (c>=4) for batch b.
    XC = np.empty((128, NCHUNK, B), dtype=BF16)
    XC[:, : NCHUNK // 2, :] = xa.T.reshape(NCHUNK // 2, 128, B).transpose(1, 0, 2)
    XC[:, NCHUNK // 2 :, :] = xb.T.reshape(NCHUNK // 2, 128, B).transpose(1, 0, 2)

    TC = np.concatenate(
        [np.asarray(weight, np.float32), np.asarray(labels, np.float32)], axis=1
    ).astype(BF16)  # [L, 1024]

    sl = np.asarray(shortlist).reshape(-1).astype(np.int64)
    core = sl // LSH
    lidx = sl % LSH
    bvec = np.repeat(np.arange(B, dtype=np.int64), S)

    # Per core: split pairs into stream (first hit of each distinct row,
    # ordered by batch) and gather (the rest, already batch-major).
    s_rows, s_b, s_pos = [], [], []   # per-core stream row ids / batches / flat pos
    g_idx, g_b, g_pos = [], [], []
    rng = np.random.default_rng(0)
    for c in range(NCORES):
        posc = np.nonzero(core == c)[0]
        li = lidx[posc]
        bv = bvec[posc]
        # Claim a RANDOM occurrence of each distinct row for the stream (the
        # first-by-batch choice would skew stream density toward low batches
        # and blow the per-tile batch window).
        is_stream = np.zeros(len(posc), bool)
        if not force_gather:
            perm = rng.permutation(len(posc))
            _, first_p = np.unique(li[perm], return_index=True)
            is_stream[perm[first_p]] = True
        first = np.nonzero(is_stream)[0]
        # stream entries: sort by (b, row) so tiles cover narrow b-windows
        sb, srow, spos = bv[first], li[first], posc[first]
        o = np.lexsort((srow, sb))
        s_rows.append(srow[o])
        s_b.append(sb[o])
        s_pos.append(spos[o])
        g_idx.append(li[~is_stream])
        g_b.append(bv[~is_stream])
        g_pos.append(posc[~is_stream])

    cap_s = int(-(-max(len(x) for x in s_rows) // TILE) * TILE)
    nstream = cap_s // TILE

    def padded_b(vals, cap):
        out = np.full(cap, -1, np.int64)
        out[: len(vals)] = vals
        return out

    blo_s = _window_schedule([padded_b(x, cap_s) for x in s_b], nstream)
    if blo_s is None:
        return None  # caller falls back to pure-gather mode

    # Layer-1 reuse: a duplicate-row pair whose batch falls inside its row's
    # stream-tile window can be answered from the streamed data via a second
    # mask pass — zero extra DMA. At most one such pair per stream slot.
    l1_slot, l1_b, l1_pos = [], [], []
    for c in range(NCORES):
        rows_g, bs_g, pos_g = g_idx[c], g_b[c], g_pos[c]
        if nstream and len(rows_g):
            slot_of_row = np.full(LSH, -1, np.int64)
            slot_of_row[s_rows[c]] = np.arange(len(s_rows[c]))
            slot = slot_of_row[rows_g]
            m = bs_g - blo_s[np.clip(slot, 0, None) // TILE]
            qual = (slot >= 0) & (m >= 0) & (m < MWIN)
            qi = np.nonzero(qual)[0]
            _, first_idx = np.unique(slot[qi], return_index=True)
            chosen = qi[first_idx]
        else:
            chosen = np.zeros(0, np.int64)
        is_l1 = np.zeros(len(rows_g), bool)
        is_l1[chosen] = True
        l1_slot.append(slot[chosen] if len(chosen) else np.zeros(0, np.int64))
        l1_b.append(bs_g[chosen])
        l1_pos.append(pos_g[chosen])
        g_idx[c] = rows_g[~is_l1]
        g_b[c] = bs_g[~is_l1]
        g_pos[c] = pos_g[~is_l1]

    cap_g = int(-(-max(1, max(len(x) for x in g_idx)) // TILE) * TILE)
    ngather = cap_g // TILE
    ntiles = nstream + ngather

    blo_g = _window_schedule([padded_b(x, cap_g) for x in g_b], ngather)
    if blo_g is None:
        return None  # caller falls back to pure-gather mode

    # Stream tables: per core [nstream, 128, NCHUNK*TILE] bf16 with
    # st[t, p, c*512+j] = TC_local[row_j, c*128+p].
    streams = []
    for c in range(NCORES):
        if nstream == 0:
            streams.append(np.zeros((1, 128, NCHUNK * TILE), dtype=BF16))
            continue
        rows = np.zeros(cap_s, np.int64)
        rows[: len(s_rows[c])] = s_rows[c]
        arr = TC[c * LSH : (c + 1) * LSH][rows]           # [cap_s, 1024]
        arr = arr.reshape(nstream, TILE, NCHUNK, 128)     # [t, j, c, p]
        streams.append(
            np.ascontiguousarray(arr.transpose(0, 3, 2, 1)).reshape(
                nstream, 128, NCHUNK * TILE
            )
        )

    idx16 = np.zeros((NCORES, cap_g), np.int16)
    maskh = np.zeros((NCORES, MWIN, ntiles * TILE), dtype=np.uint8)
    mask2h = np.zeros((NCORES, MWIN, max(nstream, 1) * TILE), dtype=np.uint8)
    for c in range(NCORES):
        n_s, n_g = len(s_b[c]), len(g_b[c])
        idx16[c, :n_g] = g_idx[c].astype(np.int16)
        ms = s_b[c] - blo_s[np.arange(n_s) // TILE]
        mg = g_b[c] - blo_g[np.arange(n_g) // TILE]
        assert (ms >= 0).all() and (ms < MWIN).all()
        assert (mg >= 0).all() and (mg < MWIN).all()
        maskh[c, ms, np.arange(n_s)] = 1
        maskh[c, mg, cap_s + np.arange(n_g)] = 1
        if len(l1_slot[c]):
            m1 = l1_b[c] - blo_s[l1_slot[c] // TILE]
            mask2h[c, m1, l1_slot[c]] = 1

    idxw = np.tile(
        idx16.reshape(NCORES, cap_g // 16, 16).transpose(0, 2, 1), (1, 8, 1)
    )  # [NCORES, 128, cap_g//16]

    in_maps = []
    ones = np.ones((MWIN, 1), dtype=BF16)
    for c in range(NCORES):
        in_maps.append(
            {
                "tc": np.ascontiguousarray(TC[c * LSH : (c + 1) * LSH]),
                "stream": streams[c],
                "xc": np.ascontiguousarray(XC.reshape(128, NCHUNK * B)),
                "idx": np.ascontiguousarray(idxw[c]),
                "mask": np.ascontiguousarray(maskh[c]),
                "mask2": np.ascontiguousarray(mask2h[c]),
                "ones": ones,
            }
        )
    # blo per global tile position is resolved after interleaving in kernel().
    meta = {
        "nstream": nstream,
        "ngather": ngather,
        "cap_s": cap_s,
        "cap_g": cap_g,
        "blo_s": blo_s,
        "blo_g": blo_g,
        "s_pos": s_pos,
        "g_pos": g_pos,
        "l1_pos": l1_pos,
        "l1_slot": l1_slot,
    }
    return in_maps, meta


def kernel(input, labels, weight, alpha, beta, bias, shortlist, _trace=False):
    from concourse.bass_utils import run_bass_kernel_spmd

    prep = _prep_inputs(input, labels, weight, alpha, beta, shortlist)
    if prep is None:
        # Stream batch-windows did not fit (unusual shortlist distribution);
        # fall back to routing every pair through dma_gather.
        prep = _prep_inputs(
            input, labels, weight, alpha, beta, shortlist, force_gather=True
        )
    assert prep is not None, "batch-window schedule failed; widen MWIN"
    in_maps, meta = prep
    nstream, ngather = meta["nstream"], meta["ngather"]

    key = (nstream, ngather)
    if key not in _PROG_CACHE:
        # The program's per-tile window bases must match the interleaved
        # order; compute order first, then blo per global tile.
        order = _tile_order(nstream, ngather)
        blo = np.array(
            [
                meta["blo_s"][k] if kind == "s" else meta["blo_g"][k]
                for kind, k in order
            ],
            np.int64,
        )
        _PROG_CACHE[key] = _build_program(nstream, ngather, blo, meta["cap_g"])
    nc, order = _PROG_CACHE[key]

    # Masks were built with stream columns first; permute to interleaved order.
    perm = np.array(
        [k if kind == "s" else nstream + k for kind, k in order], np.int64
    )
    for m in in_maps:
        mm = m["mask"].reshape(MWIN, nstream + ngather, TILE)
        m["mask"] = np.ascontiguousarray(mm[:, perm, :].reshape(MWIN, -1))

    res = run_bass_kernel_spmd(nc, in_maps, list(range(NCORES)), trace=_trace)

    out_flat = np.zeros(B * S, dtype=np.float32)
    for c in range(NCORES):
        vals = res.results[c]["out"]  # [ntiles, TILE]
        n_s = len(meta["s_pos"][c])
        n_g = len(meta["g_pos"][c])
        svals = np.empty(meta["cap_s"], np.float32)
        gvals = np.empty(meta["cap_g"], np.float32)
        for t, (kind, k) in enumerate(order):
            if kind == "s":
                svals[k * TILE : (k + 1) * TILE] = vals[t]
            else:
                gvals[k * TILE : (k + 1) * TILE] = vals[t]
        out_flat[meta["s_pos"][c]] = svals[:n_s]
        out_flat[meta["g_pos"][c]] = gvals[:n_g]
        if len(meta["l1_pos"][c]):
            vals2 = res.results[c]["out2"].reshape(-1)
            out_flat[meta["l1_pos"][c]] = vals2[meta["l1_slot"][c]]

    bias = np.asarray(bias, dtype=np.float32)
    sl = np.asarray(shortlist).reshape(-1).astype(np.int64)
    out_flat += bias[sl]
    out = out_flat.reshape(B, S)

    if _trace:
        return out, res
    return out



# revision 2
# speedup vs baseline: 1.4446x; 1.4446x over previous
"""Trainium2 Bass kernel for nn_CombineUV (shortlist-scored retrieval).

Math: out[b,s] = dot(input[b], sig(alpha)*weight[i] + sig(beta)*labels[i]) + bias[i]
with i = shortlist[b,s].

Since alpha/beta are per-feature [1,D] vectors, the host precombines the two
tables into one: CLF = sig(alpha)*weight + sig(beta)*labels  [L, D] bf16 --
halving both the per-row DMA bytes and the matmul contraction chunks versus
streaming [weight || labels].

Device strategy (8 cores, batch-sharded, pure stream):
 - Every batch b has exactly S=512 shortlist entries, so grouping pairs by
   BATCH makes each 512-pair tile exactly one batch: core c owns batches
   [c*64, (c+1)*64), tile t == batch c*64+t, columns j == shortlist column s.
 - The host pre-gathers + pre-transposes each tile's rows into a PE-ready
   [128, 4*512] bf16 block: st[t, p, c4*512+j] = CLF[shortlist[b, j], c4*128+p].
   These load with plain full-rate dma_start -- no SWDGE descriptor-gen, no
   dma_gather, and (because the tile is a single batch) no batch-window
   masking or ones-reduce matmul at all.
 - Per tile: 4 accumulating matmuls with lhsT = input[b] chunk [128, 1] give
   PSUM[1, j] = dot(input[b], CLF[i_j]) directly; scalar-copy to SBUF and DMA
   out. Host adds bias[shortlist] (O(B*S) elementwise).
 - Per-core DMA is ~32 MB of streamed rows (every pair's row, duplicates
   included) + ~0.2 MB of side data; the kernel is DMA-bound with the PE at
   ~60% occupancy underneath.
"""

import sys

sys.path.insert(0, "/opt/trn_rl_repo")

import numpy as np
import ml_dtypes

BF16 = ml_dtypes.bfloat16

L, D, B, S = 131072, 512, 512, 512
NCORES = 8
TB = B // NCORES           # batches (== tiles) per core: 64
NCHUNK = D // 128          # 4 contraction chunks of 128

_PROG = None


def _build_program():
    import concourse.bacc as bacc
    import concourse.mybir as mybir
    from concourse.tile import TileContext

    f32, bf = mybir.dt.float32, mybir.dt.bfloat16

    nc = bacc.Bacc(None, target_bir_lowering=False)
    st_d = nc.dram_tensor("st", [TB, 128, NCHUNK * S], bf, kind="ExternalInput")
    xc_d = nc.dram_tensor("xc", [128, NCHUNK * TB], bf, kind="ExternalInput")
    out_d = nc.dram_tensor("out", [TB, S], f32, kind="ExternalOutput")

    with TileContext(nc) as tc:
        with (
            tc.tile_pool(name="res", bufs=1) as res_pool,
            tc.tile_pool(name="g", bufs=8) as gpool,
            tc.tile_pool(name="o", bufs=4) as opool,
            tc.tile_pool(name="ps", bufs=6, space="PSUM") as pspool,
        ):
            xc_sb = res_pool.tile([128, NCHUNK * TB], bf, tag="xc")
            nc.sync.dma_start(out=xc_sb[:], in_=xc_d[:])

            for t in range(TB):
                g = gpool.tile([128, NCHUNK * S], bf, tag="g")
                nc.sync.dma_start(out=g[:], in_=st_d[t])
                ps = pspool.tile([1, S], f32, tag="ps")
                for c in range(NCHUNK):
                    nc.tensor.matmul(
                        out=ps[:],
                        lhsT=xc_sb[:, c * TB + t : c * TB + t + 1],
                        rhs=g[:, c * S : (c + 1) * S],
                        start=(c == 0),
                        stop=(c == NCHUNK - 1),
                    )
                ot = opool.tile([1, S], f32, tag="ot")
                nc.scalar.copy(ot[:], ps[:])
                nc.sync.dma_start(out=out_d[t : t + 1, :], in_=ot[:])

    nc.compile()
    return nc


def kernel(input, labels, weight, alpha, beta, bias, shortlist, _trace=False):
    from concourse.bass_utils import run_bass_kernel_spmd

    input = np.asarray(input, dtype=np.float32)
    alpha = np.asarray(alpha, dtype=np.float32).reshape(1, D)
    beta = np.asarray(beta, dtype=np.float32).reshape(1, D)
    sa = 1.0 / (1.0 + np.exp(-alpha))
    sb = 1.0 / (1.0 + np.exp(-beta))
    CLF = (sa * np.asarray(weight, np.float32) + sb * np.asarray(labels, np.float32)
           ).astype(BF16)                                    # [L, D]

    sl = np.asarray(shortlist).reshape(B, S).astype(np.int64)

    # Stream tiles: st[c, t, p, c4*S+j] = CLF[sl[c*TB+t, j], c4*128+p]
    R = CLF[sl.reshape(-1)]                                  # [B*S, D] bf16
    R = R.reshape(NCORES, TB, S, NCHUNK, 128)
    st = np.ascontiguousarray(R.transpose(0, 1, 4, 3, 2)).reshape(
        NCORES, TB, 128, NCHUNK * S
    )

    # lhsT columns: xc[c][p, c4*TB+t] = input[c*TB+t, c4*128+p]
    xc = np.ascontiguousarray(
        input.reshape(NCORES, TB, NCHUNK, 128).transpose(0, 3, 2, 1)
    ).astype(BF16).reshape(NCORES, 128, NCHUNK * TB)

    global _PROG
    if _PROG is None:
        _PROG = _build_program()
    nc = _PROG

    in_maps = [{"st": st[c], "xc": xc[c]} for c in range(NCORES)]
    res = run_bass_kernel_spmd(nc, in_maps, list(range(NCORES)), trace=_trace)

    out = np.concatenate([res.results[c]["out"] for c in range(NCORES)], axis=0)
    out = out.astype(np.float32)
    out += np.asarray(bias, np.float32)[sl]

    if _trace:
        return out, res
    return out


# revision 7
# speedup vs baseline: 2.4199x; 1.6752x over previous
"""Trainium2 Bass kernel for nn_CombineUV (shortlist-scored retrieval).

Math: out[b,s] = dot(input[b], sig(alpha)*weight[i] + sig(beta)*labels[i]) + bias[i]
with i = shortlist[b,s].

Since alpha/beta are per-feature [1,D] vectors, the host precombines the two
tables into one: CLF = sig(alpha)*weight + sig(beta)*labels  [L, D] bf16 --
halving both the per-row DMA bytes and the matmul contraction chunks versus
streaming [weight || labels].

Device strategy (8 cores, batch-sharded, pure stream):
 - Every batch b has exactly S=512 shortlist entries, so grouping pairs by
   BATCH makes each 512-pair tile exactly one batch: core c owns batches
   [c*64, (c+1)*64), tile t == batch c*64+t, columns j == shortlist column s.
 - The host pre-gathers + pre-transposes each tile's rows into a PE-ready
   [128, 4*512] bf16 block: st[t, p, c4*512+j] = CLF[shortlist[b, j], c4*128+p].
   These load with plain full-rate dma_start -- no SWDGE descriptor-gen, no
   dma_gather, and (because the tile is a single batch) no batch-window
   masking or ones-reduce matmul at all.
 - Per tile: 4 accumulating matmuls with lhsT = input[b] chunk [128, 1] give
   PSUM[1, j] = dot(input[b], CLF[i_j]) directly; scalar-copy to SBUF and DMA
   out. Host adds bias[shortlist] (O(B*S) elementwise).
 - Per-core DMA is ~32 MB of streamed rows (every pair's row, duplicates
   included) + ~0.2 MB of side data; the kernel is DMA-bound with the PE at
   ~60% occupancy underneath.
"""

import sys

sys.path.insert(0, "/opt/trn_rl_repo")

import numpy as np

F16 = np.float16

L, D, B, S = 131072, 512, 512, 512
NCORES = 8
TB = B // NCORES           # batches (== tiles) per core: 64
NCHUNK = D // 128          # 4 contraction chunks of 128

_PROG = None


def _build_program():
    import concourse.bacc as bacc
    import concourse.mybir as mybir
    from concourse.tile import TileContext

    f32, f16 = mybir.dt.float32, mybir.dt.float16

    nc = bacc.Bacc(None, target_bir_lowering=False)
    st_d = nc.dram_tensor("st", [TB, 128, NCHUNK * S], f16, kind="ExternalInput")
    xc_d = nc.dram_tensor("xc", [128, NCHUNK * TB], f16, kind="ExternalInput")
    out_d = nc.dram_tensor("out", [TB, S], f32, kind="ExternalOutput")

    with TileContext(nc) as tc:
        with (
            tc.tile_pool(name="res", bufs=1) as res_pool,
            tc.tile_pool(name="g", bufs=10) as gpool,
            tc.tile_pool(name="o", bufs=8) as opool,
            tc.tile_pool(name="ps", bufs=6, space="PSUM") as pspool,
        ):
            xc_sb = res_pool.tile([128, NCHUNK * TB], f16, tag="xc")
            nc.sync.dma_start(out=xc_sb[:], in_=xc_d[:])

            for t in range(TB):
                g = gpool.tile([128, NCHUNK * S], f16, tag="g")
                # Stream DMAs issue from the sync queue with no interleaved
                # waits, so the rings stay stuffed ~gpool-bufs tiles ahead.
                nc.sync.dma_start(out=g[:], in_=st_d[t])
                ps = pspool.tile([1, S], f32, tag="ps")
                for c in range(NCHUNK):
                    nc.tensor.matmul(
                        out=ps[:],
                        lhsT=xc_sb[:, c * TB + t : c * TB + t + 1],
                        rhs=g[:, c * S : (c + 1) * S],
                        start=(c == 0),
                        stop=(c == NCHUNK - 1),
                    )
                # PSUM->SBUF evacuation alternates scalar/vector; the
                # out-DMA triggers issue from the otherwise-idle gpsimd
                # queue, so their copy-done waits never block the stream-DMA
                # issue (sync) queue.
                ot = opool.tile([1, S], f32, tag="ot")
                if t % 2 == 0:
                    nc.scalar.copy(ot[:], ps[:])
                else:
                    nc.vector.tensor_copy(out=ot[:], in_=ps[:])
                nc.gpsimd.dma_start(out=out_d[t : t + 1, :], in_=ot[:])

    nc.compile()
    return nc


def kernel(input, labels, weight, alpha, beta, bias, shortlist, _trace=False):
    from concourse.bass_utils import run_bass_kernel_spmd

    input = np.asarray(input, dtype=np.float32)
    alpha = np.asarray(alpha, dtype=np.float32).reshape(1, D)
    beta = np.asarray(beta, dtype=np.float32).reshape(1, D)
    sa = 1.0 / (1.0 + np.exp(-alpha))
    sb = 1.0 / (1.0 + np.exp(-beta))
    CLF = (sa * np.asarray(weight, np.float32) + sb * np.asarray(labels, np.float32)
           ).astype(F16)                                     # [L, D]

    sl = np.asarray(shortlist).reshape(B, S).astype(np.int64)

    # Stream tiles: st[c, t, p, c4*S+j] = CLF[sl[c*TB+t, j], c4*128+p]
    R = CLF[sl.reshape(-1)]                                  # [B*S, D] bf16
    R = R.reshape(NCORES, TB, S, NCHUNK, 128)
    st = np.ascontiguousarray(R.transpose(0, 1, 4, 3, 2)).reshape(
        NCORES, TB, 128, NCHUNK * S
    )

    # lhsT columns: xc[c][p, c4*TB+t] = input[c*TB+t, c4*128+p]
    xc = np.ascontiguousarray(
        input.reshape(NCORES, TB, NCHUNK, 128).transpose(0, 3, 2, 1)
    ).astype(F16).reshape(NCORES, 128, NCHUNK * TB)

    global _PROG
    if _PROG is None:
        _PROG = _build_program()
    nc = _PROG

    in_maps = [{"st": st[c], "xc": xc[c]} for c in range(NCORES)]
    res = run_bass_kernel_spmd(nc, in_maps, list(range(NCORES)), trace=_trace)

    out = np.concatenate([res.results[c]["out"] for c in range(NCORES)], axis=0)
    out = out.astype(np.float32)
    out += np.asarray(bias, np.float32)[sl]

    if _trace:
        return out, res
    return out


# revision 11
# speedup vs baseline: 2.7217x; 1.1247x over previous
"""Trainium2 Bass kernel for nn_CombineUV (shortlist-scored retrieval).

Math: out[b,s] = dot(input[b], sig(alpha)*weight[i] + sig(beta)*labels[i]) + bias[i]
with i = shortlist[b,s].

Since alpha/beta are per-feature [1,D] vectors, the host precombines the two
tables into one: CLF = sig(alpha)*weight + sig(beta)*labels  [L, D] bf16 --
halving both the per-row DMA bytes and the matmul contraction chunks versus
streaming [weight || labels].

Device strategy (8 cores, batch-sharded, pure stream):
 - Every batch b has exactly S=512 shortlist entries, so grouping pairs by
   BATCH makes each 512-pair tile exactly one batch: core c owns batches
   [c*64, (c+1)*64), tile t == batch c*64+t, columns j == shortlist column s.
 - The host pre-gathers + pre-transposes each tile's rows into a PE-ready
   [128, 4*512] bf16 block: st[t, p, c4*512+j] = CLF[shortlist[b, j], c4*128+p].
   These load with plain full-rate dma_start -- no SWDGE descriptor-gen, no
   dma_gather, and (because the tile is a single batch) no batch-window
   masking or ones-reduce matmul at all.
 - Per tile: 4 accumulating matmuls with lhsT = input[b] chunk [128, 1] give
   PSUM[1, j] = dot(input[b], CLF[i_j]) directly; scalar-copy to SBUF and DMA
   out. Host adds bias[shortlist] (O(B*S) elementwise).
 - Per-core DMA is ~32 MB of streamed rows (every pair's row, duplicates
   included) + ~0.2 MB of side data; the kernel is DMA-bound with the PE at
   ~60% occupancy underneath.
"""

import sys

sys.path.insert(0, "/opt/trn_rl_repo")

import numpy as np

F16 = np.float16

L, D, B, S = 131072, 512, 512, 512
NCORES = 8
TB = B // NCORES           # batches (== tiles) per core: 64
NCHUNK = D // 128          # 4 contraction chunks of 128
GB = 4                     # batches per DMA group (2MB stream transfers)

_PROG = None


def _build_program():
    import concourse.bacc as bacc
    import concourse.mybir as mybir
    from concourse.tile import TileContext

    f32, f16 = mybir.dt.float32, mybir.dt.float16
    NG = TB // GB                       # DMA groups per core

    nc = bacc.Bacc(None, target_bir_lowering=False)
    st_d = nc.dram_tensor("st", [NG, 128, GB * NCHUNK * S], f16, kind="ExternalInput")
    xc_d = nc.dram_tensor("xc", [128, NCHUNK * TB], f16, kind="ExternalInput")
    out_d = nc.dram_tensor("out", [NG, GB * S], f32, kind="ExternalOutput")

    with TileContext(nc) as tc:
        with (
            tc.tile_pool(name="res", bufs=1) as res_pool,
            tc.tile_pool(name="g", bufs=5) as gpool,
            tc.tile_pool(name="o", bufs=4) as opool,
            tc.tile_pool(name="ps", bufs=8, space="PSUM") as pspool,
        ):
            xc_sb = res_pool.tile([128, NCHUNK * TB], f16, tag="xc")
            nc.sync.dma_start(out=xc_sb[:], in_=xc_d[:])

            for gi in range(NG):
                g = gpool.tile([128, GB * NCHUNK * S], f16, tag="g")
                # Stream DMAs issue from the sync queue with no interleaved
                # waits, so the rings stay stuffed ~gpool-bufs groups ahead.
                # 16KB per partition per transfer keeps descriptors big.
                nc.sync.dma_start(out=g[:], in_=st_d[gi])
                ot = opool.tile([1, GB * S], f32, tag="ot")
                for k in range(GB):
                    t = gi * GB + k
                    ps = pspool.tile([1, S], f32, tag="ps")
                    for c in range(NCHUNK):
                        nc.tensor.matmul(
                            out=ps[:],
                            lhsT=xc_sb[:, c * TB + t : c * TB + t + 1],
                            rhs=g[:, (k * NCHUNK + c) * S : (k * NCHUNK + c + 1) * S],
                            start=(c == 0),
                            stop=(c == NCHUNK - 1),
                        )
                    # PSUM->SBUF evacuation alternates scalar/vector into a
                    # per-group output strip; one out-DMA per group issues
                    # from the otherwise-idle gpsimd queue, so its copy-done
                    # wait never blocks the stream-DMA issue (sync) queue.
                    if t % 2 == 0:
                        nc.scalar.copy(ot[:, k * S : (k + 1) * S], ps[:])
                    else:
                        nc.vector.tensor_copy(out=ot[:, k * S : (k + 1) * S], in_=ps[:])
                nc.gpsimd.dma_start(out=out_d[gi : gi + 1, :], in_=ot[:])

    nc.compile()
    return nc


def kernel(input, labels, weight, alpha, beta, bias, shortlist, _trace=False):
    from concourse.bass_utils import run_bass_kernel_spmd

    input = np.asarray(input, dtype=np.float32)
    alpha = np.asarray(alpha, dtype=np.float32).reshape(1, D)
    beta = np.asarray(beta, dtype=np.float32).reshape(1, D)
    sa = 1.0 / (1.0 + np.exp(-alpha))
    sb = 1.0 / (1.0 + np.exp(-beta))
    CLF = (sa * np.asarray(weight, np.float32) + sb * np.asarray(labels, np.float32)
           ).astype(F16)                                     # [L, D]

    sl = np.asarray(shortlist).reshape(B, S).astype(np.int64)

    # Stream tiles: st[c, gi, p, (k*NCHUNK+c4)*S+j] = CLF[sl[c*TB+gi*GB+k, j],
    # c4*128+p] -- GB batches per DMA group, PE-ready transposed layout.
    R = CLF[sl.reshape(-1)]                                  # [B*S, D] f16
    R = R.reshape(NCORES, TB // GB, GB, S, NCHUNK, 128)
    st = np.ascontiguousarray(R.transpose(0, 1, 5, 2, 4, 3)).reshape(
        NCORES, TB // GB, 128, GB * NCHUNK * S
    )

    # lhsT columns: xc[c][p, c4*TB+t] = input[c*TB+t, c4*128+p]
    xc = np.ascontiguousarray(
        input.reshape(NCORES, TB, NCHUNK, 128).transpose(0, 3, 2, 1)
    ).astype(F16).reshape(NCORES, 128, NCHUNK * TB)

    global _PROG
    if _PROG is None:
        _PROG = _build_program()
    nc = _PROG

    in_maps = [{"st": st[c], "xc": xc[c]} for c in range(NCORES)]
    res = run_bass_kernel_spmd(nc, in_maps, list(range(NCORES)), trace=_trace)

    out = np.concatenate(
        [res.results[c]["out"].reshape(TB, S) for c in range(NCORES)], axis=0
    )
    out = out.astype(np.float32)
    out += np.asarray(bias, np.float32)[sl]

    if _trace:
        return out, res
    return out


# revision 14
# speedup vs baseline: 2.8299x; 1.0398x over previous
"""Trainium2 Bass kernel for nn_CombineUV (shortlist-scored retrieval).

Math: out[b,s] = dot(input[b], sig(alpha)*weight[i] + sig(beta)*labels[i]) + bias[i]
with i = shortlist[b,s].

Since alpha/beta are per-feature [1,D] vectors, the host precombines the two
tables into one: CLF = sig(alpha)*weight + sig(beta)*labels  [L, D] bf16 --
halving both the per-row DMA bytes and the matmul contraction chunks versus
streaming [weight || labels].

Device strategy (8 cores, batch-sharded, pure stream):
 - Every batch b has exactly S=512 shortlist entries, so grouping pairs by
   BATCH makes each 512-pair tile exactly one batch: core c owns batches
   [c*64, (c+1)*64), tile t == batch c*64+t, columns j == shortlist column s.
 - The host pre-gathers + pre-transposes each tile's rows into a PE-ready
   [128, 4*512] bf16 block: st[t, p, c4*512+j] = CLF[shortlist[b, j], c4*128+p].
   These load with plain full-rate dma_start -- no SWDGE descriptor-gen, no
   dma_gather, and (because the tile is a single batch) no batch-window
   masking or ones-reduce matmul at all.
 - Per tile: 4 accumulating matmuls with lhsT = input[b] chunk [128, 1] give
   PSUM[1, j] = dot(input[b], CLF[i_j]) directly; scalar-copy to SBUF and DMA
   out. Host adds bias[shortlist] (O(B*S) elementwise).
 - Per-core DMA is ~32 MB of streamed rows (every pair's row, duplicates
   included) + ~0.2 MB of side data; the kernel is DMA-bound with the PE at
   ~60% occupancy underneath.
"""

import sys

sys.path.insert(0, "/opt/trn_rl_repo")

import numpy as np

F16 = np.float16

L, D, B, S = 131072, 512, 512, 512
NCORES = 8
TB = B // NCORES           # batches (== tiles) per core: 64
NCHUNK = D // 128          # 4 contraction chunks of 128
GDMA = 2                   # batches per stream DMA (1MB transfers)
GOUT = 8                   # batches per output strip / out-DMA

_PROG = None


def _build_program():
    import concourse.bacc as bacc
    import concourse.mybir as mybir
    from concourse.tile import TileContext

    f32, f16 = mybir.dt.float32, mybir.dt.float16
    ND = TB // GDMA                     # stream DMAs per core
    NO = TB // GOUT                     # output strips per core

    nc = bacc.Bacc(None, target_bir_lowering=False)
    st_d = nc.dram_tensor("st", [ND, 128, GDMA * NCHUNK * S], f16, kind="ExternalInput")
    xc_d = nc.dram_tensor("xc", [128, NCHUNK * TB], f16, kind="ExternalInput")
    out_d = nc.dram_tensor("out", [NO, GOUT * S], f32, kind="ExternalOutput")

    with TileContext(nc) as tc:
        with (
            tc.tile_pool(name="res", bufs=1) as res_pool,
            tc.tile_pool(name="g", bufs=10) as gpool,
            tc.tile_pool(name="o", bufs=3) as opool,
            tc.tile_pool(name="ps", bufs=8, space="PSUM") as pspool,
        ):
            xc_sb = res_pool.tile([128, NCHUNK * TB], f16, tag="xc")
            nc.sync.dma_start(out=xc_sb[:], in_=xc_d[:])

            ot = None
            for t in range(TB):
                k2 = t % GDMA
                if k2 == 0:
                    g = gpool.tile([128, GDMA * NCHUNK * S], f16, tag="g")
                    # Stream DMAs issue from the sync queue with no
                    # interleaved waits, so the rings stay stuffed
                    # ~gpool-bufs transfers ahead.
                    nc.sync.dma_start(out=g[:], in_=st_d[t // GDMA])
                ko = t % GOUT
                if ko == 0:
                    ot = opool.tile([1, GOUT * S], f32, tag="ot")
                ps = pspool.tile([1, S], f32, tag="ps")
                for c in range(NCHUNK):
                    nc.tensor.matmul(
                        out=ps[:],
                        lhsT=xc_sb[:, c * TB + t : c * TB + t + 1],
                        rhs=g[:, (k2 * NCHUNK + c) * S : (k2 * NCHUNK + c + 1) * S],
                        start=(c == 0),
                        stop=(c == NCHUNK - 1),
                    )
                # PSUM->SBUF evacuation alternates scalar/vector into a
                # per-strip output buffer; one out-DMA per strip issues from
                # the otherwise-idle gpsimd queue, so its copy-done wait
                # never blocks the stream-DMA issue (sync) queue.
                if t % 2 == 0:
                    nc.scalar.copy(ot[:, ko * S : (ko + 1) * S], ps[:])
                else:
                    nc.vector.tensor_copy(out=ot[:, ko * S : (ko + 1) * S], in_=ps[:])
                if ko == GOUT - 1:
                    nc.gpsimd.dma_start(
                        out=out_d[t // GOUT : t // GOUT + 1, :], in_=ot[:]
                    )

    nc.compile()
    return nc


def kernel(input, labels, weight, alpha, beta, bias, shortlist, _trace=False):
    from concourse.bass_utils import run_bass_kernel_spmd

    input = np.asarray(input, dtype=np.float32)
    alpha = np.asarray(alpha, dtype=np.float32).reshape(1, D)
    beta = np.asarray(beta, dtype=np.float32).reshape(1, D)
    sa = 1.0 / (1.0 + np.exp(-alpha))
    sb = 1.0 / (1.0 + np.exp(-beta))
    CLF = (sa * np.asarray(weight, np.float32) + sb * np.asarray(labels, np.float32)
           ).astype(F16)                                     # [L, D]

    sl = np.asarray(shortlist).reshape(B, S).astype(np.int64)

    # Stream tiles: st[c, di, p, (k*NCHUNK+c4)*S+j] = CLF[sl[c*TB+di*GDMA+k, j],
    # c4*128+p] -- GDMA batches per stream DMA, PE-ready transposed layout.
    R = CLF[sl.reshape(-1)]                                  # [B*S, D] f16
    R = R.reshape(NCORES, TB // GDMA, GDMA, S, NCHUNK, 128)
    st = np.ascontiguousarray(R.transpose(0, 1, 5, 2, 4, 3)).reshape(
        NCORES, TB // GDMA, 128, GDMA * NCHUNK * S
    )

    # lhsT columns: xc[c][p, c4*TB+t] = input[c*TB+t, c4*128+p]
    xc = np.ascontiguousarray(
        input.reshape(NCORES, TB, NCHUNK, 128).transpose(0, 3, 2, 1)
    ).astype(F16).reshape(NCORES, 128, NCHUNK * TB)

    global _PROG
    if _PROG is None:
        _PROG = _build_program()
    nc = _PROG

    in_maps = [{"st": st[c], "xc": xc[c]} for c in range(NCORES)]
    res = run_bass_kernel_spmd(nc, in_maps, list(range(NCORES)), trace=_trace)

    out = np.concatenate(
        [res.results[c]["out"].reshape(TB, S) for c in range(NCORES)], axis=0
    )
    out = out.astype(np.float32)
    out += np.asarray(bias, np.float32)[sl]

    if _trace:
        return out, res
    return out
